# revision 23
# baseline (speedup 1.0000x reference)
"""Trainium2 Bass kernel for a DoReFa-quantized ResNet BasicBlock (inference).

Reference computation (all fp32):
    out = qact(bn2(conv3x3(qact(bn1(conv3x3(x, qw(w1)))), qw(w2))) + x)
with qw = 4-bit DoReFa weight quant, qact = 4-bit activation quant,
x: (64, 128, 56, 56), convs 128->128 stride 1 pad 1.

Sharding: data-parallel over the batch dim, 8 images per NeuronCore on 8 cores.

Per-core kernel design:
  * HW truth (traced + microbenched): every matmul streams ~1 output element
    per cycle at 2.4 GHz regardless of dtype or perf-mode (fp8 DoubleRow /
    DoublePixel / fp16 all ~189ns for 448 free elems; the cost model's 0.5
    cyc/row for DR is wrong on this silicon).  DoubleRow's real win is that
    one DR matmul consumes TWO taps (doubled contraction).  PE floor =
    (9 conv1 + 5 conv2 matmuls) x 7 chunks x 448 x 8 img ~ 148 us; the
    whole kernel is built to keep that stream gap-free.
  * conv1: fp16 matmuls straight from a host-padded fp16 15*x plane (58x58
    rows, zero borders): 9 full matmuls per 8-row PSUM chunk, no on-chip
    padding work, conv1 error ~2^-11.  The same plane is the residual.
  * conv2: fp8e4m3 exact integer arithmetic (act 15a in {0..15}, weights
    15w odd in [-15,15]): 1 center matmul + 3 DoubleRow (dy=-1/+1 pairs,
    128B pair stride) + 1 DR pairing (dy=0,dx=-1) with (dx=+1) read from a
    shifted act1 duplicate at +ACT_D (pair stride 3728 % 16 == 0).
  * bn1 folds to Relu(s*psum+b) on ScalarE writing fp16 v1 (lower clip
    free); DVE min15 (4x mode) then two +2^23 round-ops write the two fp8
    act1 copies.  bn2 writes fp16 v2; tail per image piece: fp16 residual
    add, 4x round-op, clamp written as fp8 ints {0..15}; the host upcasts
    and divides by 15 (exact).  Per-image DMA: 6.7KB in + 3.1KB out per
    partition.
  * Edge matmuls are trimmed to nonzero products: conv1 taps drop the rows/
    cols whose input is a zero pad (center tap first carries start=True full
    coverage; a full dx=0 tap carries stop), and conv2's dy=+-1 DR pairs at
    dx=+-1 drop the one output column where both members read the pad
    column (~2.5 us of pure-zero PE work removed).
  * Software-pipelined emission (conv1 of image n+1 ahead of conv2 of n);
    act1 double-buffered as two static tiles with pad borders zeroed once;
    triple-buffered SBUF pools kill image-boundary WAR stalls; the round
    chain runs in image halves so conv2's early chunks unblock sooner;
    image 0's input DMA is split into row bands; the last output piece is
    a single 8-row chunk so the post-stream drain is short.  A post-Tile
    pass splits multi-semaphore waits onto same-engine NoOps (walrus
    encodes at most one sync wait per instruction).
  * Fixed overheads measured: ~8.5 us DMA-engine/queue startup before the
    first input byte moves, and a ~7 us NEFF exit epilogue (~60 rounds of
    EVENT_SEMAPHORE cleanup across all sequencers) — both invariant to
    kernel structure.  Occasional runs show ~17% uniform PE-clock
    throttling (external to the kernel).

Measured (8 cores, NTFF profile): 170.5-172.8 us HW exec (baseline
f32r/fp32 kernel: ~232 us on the same setup), rel L2 err 0.0106, PE
stream 100% dense for its entire ~151 us window.
"""

import os
import sys

import numpy as np

for _p in ("/opt/trn_rl_repo", "/opt/pypackages"):
    if _p not in sys.path and os.path.isdir(_p):
        sys.path.insert(0, _p)

import ml_dtypes  # noqa: E402

# ---------------------------------------------------------------- constants
B, C, H, W = 64, 128, 56, 56
N_CORES = 8
BPC = B // N_CORES          # images per core
RPC = 8                     # output rows per PSUM chunk
NCHUNK = H // RPC           # 7 chunks
FREE = RPC * W              # 448 PSUM elems per chunk
XP = H + 2                  # padded x row length (58)
XPLANE = XP * XP            # 3364 fp16 elems per padded x plane
AW = 64                     # act1 padded row pitch (bytes, fp8)
AIMG = (H + 2) * AW         # 3712 padded act1 plane bytes
AB = 1                      # act1 base offset: keeps round-op dst offsets even
ACT_D = 3726                # shifted act1 copy offset; pair step +2 % 16 == 0
ATW = ACT_D + AB + AIMG     # act1 tile width (orig + shifted copy)
PLANE = H * W               # 3136
QUAD = PLANE // 4           # 784 elems per output quarter
MAGIC = float(2**23)        # fp32 round-to-nearest-even magic constant
EPS = 1e-5

_CACHE = {}


# ---------------------------------------------------------------- host math
def _quant_weight_int(w):
    """Return 15*quantize_weight(w, 4): exact odd integers in [-15, 15]."""
    wt = np.tanh(w.astype(np.float64)).astype(np.float32)
    m = np.float32(np.abs(wt).max())
    wtn = wt / (np.float32(2.0) * m) + np.float32(0.5)
    q = np.round(wtn * np.float32(15.0)).astype(np.float32)
    return np.float32(2.0) * q - np.float32(15.0)


def _bn_affine(gamma, beta, mean, var):
    inv = 1.0 / np.sqrt(var.astype(np.float64) + EPS)
    s = gamma.astype(np.float64) * inv
    b = beta.astype(np.float64) - mean.astype(np.float64) * s
    return s, b


def _lhsT_taps(w_int):
    """[oc, ic, 3, 3] -> [ic, 9*oc] stationary layout, tap-major."""
    t = np.transpose(w_int, (2, 3, 1, 0)).reshape(9, C, C)   # [tap, ic, oc]
    return np.transpose(t, (1, 0, 2)).reshape(C, 9 * C)


# ---------------------------------------------------------------- bass build
def _split_multiwaits(nc, mybir):
    """Walrus encodes at most ONE sync wait per instruction; hoist extras
    onto same-engine NoOps placed immediately before."""
    nid = 0
    for fn in nc.m.functions:
        for blk in fn.blocks:
            out = []
            changed = False
            for ins in blk.instructions:
                si = ins.sync_info
                if si is not None and len(si.on_wait) > 1:
                    waits = list(si.on_wait)
                    for w in waits[:-1]:
                        nid += 1
                        nop = mybir.InstNoOp(name=f"I-wfix-{nid}",
                                             engine=ins.engine)
                        nop.sync_info = mybir.SyncInfo(on_wait=[w],
                                                       on_update=[])
                        out.append(nop)
                    ins.sync_info = mybir.SyncInfo(
                        on_wait=[waits[-1]], on_update=list(si.on_update))
                    changed = True
                out.append(ins)
            if changed:
                blk.instructions = out


def _build_module(apply_wfix=True):
    import concourse.bass as bass
    import concourse.mybir as mybir
    import concourse.tile as tile
    from contextlib import ExitStack

    f32 = mybir.dt.float32
    f16 = mybir.dt.float16
    f8 = mybir.dt.float8e4
    AF = mybir.ActivationFunctionType
    OP = mybir.AluOpType
    DR = mybir.MatmulPerfMode.DoubleRow

    nc = bass.Bass("TRN2", target_bir_lowering=False, debug=False,
                   num_devices=N_CORES)

    xf_d = nc.dram_tensor("xf", [BPC, C, XPLANE], f16, kind="ExternalInput")
    w1_d = nc.dram_tensor("w1p", [C, 9 * C], f16, kind="ExternalInput")
    w2_d = nc.dram_tensor("w2p", [C, 9 * C], f8, kind="ExternalInput")
    bn_d = nc.dram_tensor("bnv", [C, 4], f32, kind="ExternalInput")
    out_d = nc.dram_tensor("out", [BPC, C, PLANE], f8, kind="ExternalOutput")

    with tile.TileContext(nc) as tc, ExitStack() as ctx:
        const = ctx.enter_context(tc.tile_pool(name="const", bufs=1))
        sb = ctx.enter_context(tc.tile_pool(name="sb", bufs=3))
        ps = ctx.enter_context(tc.tile_pool(name="ps", bufs=4, space="PSUM"))

        # conv1 weights first: they gate image 0's first matmul
        w1_sb = const.tile([C, 9 * C], f16)
        nc.sync.dma_start(w1_sb[:], w1_d.ap())
        bn_sb = const.tile([C, 4], f32)
        w2_sb = const.tile([C, 9 * C], f8)
        sc1, bi1 = bn_sb[:, 0:1], bn_sb[:, 1:2]
        sc2, bi2 = bn_sb[:, 2:3], bn_sb[:, 3:4]

        # PE p-state warm-up: the tensor engine needs ~3us of continuous
        # work to reach 2.4GHz; run short dummy matmuls on zeroed data
        # during the input-DMA wait so image 0 starts at full clock
        warm = const.tile([C, 512], f8)
        nc.gpsimd.memset(warm[:], 0.0)
        wps = ps.tile([C, 112], f32, tag="p2", name="warm_ps",
                      bufs=3)
        for k in range(30):
            mv = bass.AP(tensor=warm.tensor, offset=0,
                         ap=[[512, C], [1, 112]])
            nc.tensor.matmul(wps[:], lhsT=warm[:, 128:256], rhs=mv,
                             start=(k == 0), stop=(k == 29))

        # two static act1 slots; pad borders zeroed once
        act1a = const.tile([C, ATW], f8)
        act1b = const.tile([C, ATW], f8)
        for a1 in (act1a, act1b):
            for base in (AB, ACT_D + AB):
                r = a1[:, base:base + AIMG].rearrange("p (h w) -> p h w", w=AW)
                nc.gpsimd.memset(r[:, 0:1, :], 0.0)        # pad row 0
                nc.gpsimd.memset(r[:, 57:58, :], 0.0)      # pad row 57
                nc.gpsimd.memset(r[:, 1:57, 0], 0.0)       # pad col 0
                nc.gpsimd.memset(r[:, 1:57, 57:64], 0.0)   # dead cols

        def emit_load_conv1(n):
            """Load image n, conv1 + bn1 + qact; returns (xf, act1-slot)."""
            xf = sb.tile([C, XPLANE], f16, tag="xf", name=f"xf_{n}")
            if n == 0:
                # 4 row bands so chunk 0's matmuls start ~3us earlier
                for b0 in range(0, XPLANE, 16 * XP):
                    b1 = min(b0 + 16 * XP, XPLANE)
                    nc.sync.dma_start(xf[:, b0:b1], xf_d.ap()[n][:, b0:b1])
                # deferred const loads: off image 0's first-matmul path
                nc.sync.dma_start(bn_sb[:], bn_d.ap())
                nc.sync.dma_start(w2_sb[:], w2_d.ap())
            else:
                nc.sync.dma_start(xf[:], xf_d.ap()[n])

            v1 = sb.tile([C, PLANE], f16, tag="v1", name=f"v1_{n}")
            a1 = act1a if n % 2 == 0 else act1b

            for cch in range(NCHUNK):
                r0 = RPC * cch
                if cch == 5:
                    emit_round_half(n, v1, a1, 0)
                p1 = ps.tile([C, FREE], f32, tag="p1", name=f"p1_{n}_{cch}",
                             bufs=5)
                # center tap first (start, full 448); a full-coverage dx=0
                # tap last (stop); edge taps trimmed to nonzero products
                last = (1, 0) if cch < NCHUNK - 1 else (-1, 0)
                taps = [(0, 0)] + [(dy, dx) for dy in (-1, 0, 1)
                                   for dx in (-1, 0, 1)
                                   if (dy, dx) != (0, 0) and (dy, dx) != last]
                taps.append(last)
                for i, (dy, dx) in enumerate(taps):
                    t9 = (dy + 1) * 3 + (dx + 1)
                    rlo = max(r0, -dy)
                    rhi = min(r0 + RPC - 1, H - 1 - dy)
                    nr = rhi - rlo + 1
                    j0 = 0 if dx >= 0 else 1
                    nj = W - abs(dx)
                    off = (rlo + dy + 1) * XP + 1 + j0 + dx
                    mv = bass.AP(tensor=xf.tensor, offset=off,
                                 ap=[[XPLANE, C], [XP, nr], [1, nj]])
                    out = bass.AP(tensor=p1.tensor,
                                  offset=(rlo - r0) * W + j0,
                                  ap=[[FREE, C], [W, nr], [1, nj]])
                    nc.tensor.matmul(out, lhsT=w1_sb[:, t9 * C:(t9 + 1) * C],
                                     rhs=mv, start=(i == 0), stop=(i == 8))
                # bn1: Relu(s*psum+b) -> v1 fp16 (lower clip for free)
                nc.scalar.activation(v1[:, FREE * cch:FREE * (cch + 1)],
                                     p1[:], AF.Relu, bias=bi1, scale=sc1)

            return v1, xf, a1

        def emit_round_half(n, v1, a1, h):
            # upper clip + round-to-int into both fp8 act1 copies, by half:
            # conv2's early chunks unblock as soon as half 0 lands
            r0, r1 = (0, 28) if h == 0 else (28, 56)
            sl = slice(W * r0, W * r1)
            nc.vector.tensor_scalar_min(v1[:, sl], v1[:, sl], 15.0)
            v1r = v1[:].rearrange("p (h w) -> p h w", w=W)
            for base in (AB, ACT_D + AB):
                ar = a1[:, base:base + AIMG].rearrange("p (h w) -> p h w",
                                                       w=AW)
                nc.vector.tensor_scalar(ar[:, r0 + 1:r1 + 1, 1:57],
                                        v1r[:, r0:r1, :], MAGIC, MAGIC,
                                        op0=OP.add, op1=OP.subtract)

        def emit_conv2_out(n, xf, a1):
            """conv2 + bn2 + residual + qact for image n, DMA fp8 ints out."""
            v2 = sb.tile([C, PLANE], f16, tag="v2", name=f"v2_{n}")
            ost = sb.tile([C, PLANE], f8, tag="ost", name=f"ost_{n}")
            od = out_d.ap()[n]
            xfr = xf[:].rearrange("p (h w) -> p h w", w=XP)

            def emit_piece(r0, r1):
                sl = slice(W * r0, W * r1)
                nc.vector.tensor_tensor(v2[:, sl], v2[:, sl],
                                        xfr[:, r0 + 1:r1 + 1, 1:57],
                                        op=OP.add)
                nc.vector.tensor_scalar(v2[:, sl], v2[:, sl], MAGIC, MAGIC,
                                        op0=OP.add, op1=OP.subtract)
                nc.vector.tensor_scalar(ost[:, sl], v2[:, sl], 0.0, 15.0,
                                        op0=OP.max, op1=OP.min)
                nc.sync.dma_start(od[:, sl], ost[:, sl])

            # last piece is a single chunk so the post-stream drain is short
            piece_after = {1: (0, 14), 3: (14, 28), 5: (28, 48), 6: (48, 56)}
            for cch in range(NCHUNK):
                r0 = RPC * cch
                p2 = ps.tile([C, FREE], f32, tag="p2", name=f"p2_{n}_{cch}",
                             bufs=3)
                # center single (start, full coverage)
                mv = bass.AP(tensor=a1.tensor,
                             offset=AB + (r0 + 1) * AW + 1,
                             ap=[[ATW, C], [AW, RPC], [1, W]])
                nc.tensor.matmul(p2[:], lhsT=w2_sb[:, 8 * C:9 * C], rhs=mv,
                                 start=True, stop=False)
                # (dy=-1,dx)+(dy=+1,dx) DR pairs, stride 2*AW = 128B;
                # for dx=+-1 both members read the same zero pad column, so
                # that output column is trimmed
                for dxi, dx in enumerate((-1, 0, 1)):
                    j0 = 0 if dx >= 0 else 1
                    nj = W - abs(dx)
                    mv = bass.AP(tensor=a1.tensor,
                                 offset=AB + r0 * AW + 1 + j0 + dx,
                                 ap=[[ATW, C], [2 * AW, 2], [AW, RPC],
                                     [1, nj]])
                    wpair = w2_sb[:, dxi * 2 * C:(dxi + 1) * 2 * C].rearrange(
                        "p (two m) -> p two m", two=2)
                    out = bass.AP(tensor=p2.tensor, offset=j0,
                                  ap=[[FREE, C], [W, RPC], [1, nj]])
                    nc.tensor.matmul(out, lhsT=wpair, rhs=mv, perf_mode=DR,
                                     start=False, stop=False)
                # (dy=0,dx=-1)@orig + (dy=0,dx=+1)@shifted, stride ACT_D+2
                mv = bass.AP(tensor=a1.tensor, offset=AB + (r0 + 1) * AW,
                             ap=[[ATW, C], [ACT_D + 2, 2], [AW, RPC],
                                 [1, W]])
                wpair = w2_sb[:, 6 * C:8 * C].rearrange(
                    "p (two m) -> p two m", two=2)
                nc.tensor.matmul(p2[:], lhsT=wpair, rhs=mv, perf_mode=DR,
                                 start=False, stop=True)
                nc.scalar.activation(v2[:, FREE * cch:FREE * (cch + 1)],
                                     p2[:], AF.Identity, bias=bi2, scale=sc2)
                if cch in piece_after:
                    emit_piece(*piece_after[cch])

        prev = None
        for s in range(BPC + 1):
            cur = emit_load_conv1(s) if s < BPC else None
            if cur is not None:
                emit_round_half(s, cur[0], cur[2], 1)
            if prev is not None:
                emit_conv2_out(s - 1, prev[1], prev[2])
            prev = cur

    if apply_wfix:
        _split_multiwaits(nc, mybir)
    return nc


def _get_module(apply_wfix=True):
    key = ("nc", apply_wfix)
    if key not in _CACHE:
        _CACHE[key] = _build_module(apply_wfix)
    return _CACHE[key]


# ---------------------------------------------------------------- host entry
def _make_in_maps(x, w1, w2, gamma1, beta1, mean1, var1,
                  gamma2, beta2, mean2, var2):
    F8 = ml_dtypes.float8_e4m3
    x15 = np.float32(15.0) * np.asarray(x, np.float32)
    x15 = x15.reshape(N_CORES, BPC, C, H, W)

    # padded fp16 plane: rows/cols 1..56 live, zero borders
    xf = np.zeros((N_CORES, BPC, C, XP, XP), np.float16)
    xf[..., 1:57, 1:57] = x15.astype(np.float16)
    xf = xf.reshape(N_CORES, BPC, C, XPLANE)

    w1i = _quant_weight_int(np.asarray(w1, np.float32))
    w2i = _quant_weight_int(np.asarray(w2, np.float32))
    w1p = _lhsT_taps(w1i).astype(np.float16)
    w2t = _lhsT_taps(w2i)
    tap2 = lambda t9: w2t[:, t9 * C:(t9 + 1) * C]
    # conv2: 3 (dy=-1,dx)+(dy=+1,dx) pairs, the dy=0 dx=-1/+1 pair, center
    blocks = []
    for dxi in range(3):
        blocks += [tap2(dxi), tap2(6 + dxi)]
    blocks += [tap2(3), tap2(5), tap2(4)]
    w2p = np.concatenate(blocks, axis=1).astype(F8)

    s1, b1 = _bn_affine(np.asarray(gamma1, np.float32),
                        np.asarray(beta1, np.float32),
                        np.asarray(mean1, np.float32),
                        np.asarray(var1, np.float32))
    s2, b2 = _bn_affine(np.asarray(gamma2, np.float32),
                        np.asarray(beta2, np.float32),
                        np.asarray(mean2, np.float32),
                        np.asarray(var2, np.float32))
    # PSUM holds 225*conv (15x and 15w) -> affine to 15*bn
    bnv = np.stack([s1 / 15.0, 15.0 * b1, s2 / 15.0, 15.0 * b2],
                   axis=1).astype(np.float32)

    shared = {"w1p": w1p, "w2p": w2p, "bnv": bnv}
    return [{"xf": np.ascontiguousarray(xf[i]), **shared}
            for i in range(N_CORES)]


def kernel(**inputs):
    from concourse.bass_utils import run_bass_kernel_spmd

    nc = _get_module()
    in_maps = _make_in_maps(**inputs)
    res = run_bass_kernel_spmd(nc, in_maps, core_ids=list(range(N_CORES)))
    _CACHE["last_res"] = res
    # exact: out fp8 ints k in 0..15 -> f32 k/15
    out = np.concatenate(
        [np.asarray(r["out"]).astype(np.float32) / np.float32(15.0)
         for r in res.results], axis=0)
    return out.reshape(B, C, H, W)


# revision 24
# speedup vs baseline: 1.0194x; 1.0194x over previous
"""Trainium2 Bass kernel for a DoReFa-quantized ResNet BasicBlock (inference).

Reference computation (all fp32):
    out = qact(bn2(conv3x3(qact(bn1(conv3x3(x, qw(w1)))), qw(w2))) + x)
with qw = 4-bit DoReFa weight quant, qact = 4-bit activation quant,
x: (64, 128, 56, 56), convs 128->128 stride 1 pad 1.

Sharding: data-parallel over the batch dim, 8 images per NeuronCore on 8 cores.

Per-core kernel design:
  * HW truth (traced + microbenched): every matmul streams ~1 output element
    per cycle at 2.4 GHz regardless of dtype or perf-mode (fp8 DoubleRow /
    DoublePixel / fp16 all ~189ns for 448 free elems; the cost model's 0.5
    cyc/row for DR is wrong on this silicon).  DoubleRow's real win is that
    one DR matmul consumes TWO taps (doubled contraction).  PE floor =
    (9 conv1 + 5 conv2 matmuls) x 7 chunks x 448 x 8 img ~ 148 us; the
    whole kernel is built to keep that stream gap-free.
  * conv1: fp16 matmuls straight from a host-padded fp16 15*x plane (58x58
    rows, zero borders): 9 full matmuls per 8-row PSUM chunk, no on-chip
    padding work, conv1 error ~2^-11.  The same plane is the residual.
  * conv2: fp8e4m3 exact integer arithmetic (act 15a in {0..15}, weights
    15w odd in [-15,15]): 1 center matmul + 3 DoubleRow (dy=-1/+1 pairs,
    128B pair stride) + 1 DR pairing (dy=0,dx=-1) with (dx=+1) read from a
    shifted act1 duplicate at +ACT_D (pair stride 3728 % 16 == 0).
  * bn1 folds to Relu(s*psum+b) on ScalarE writing fp16 v1 (lower clip
    free); DVE min15 (4x mode) then two +2^23 round-ops write the two fp8
    act1 copies.  bn2 writes fp16 v2; tail per image piece: fp16 residual
    add, 4x round-op, clamp written as fp8 ints {0..15}; the host upcasts
    and divides by 15 (exact).  Per-image DMA: 6.7KB in + 3.1KB out per
    partition.
  * Edge matmuls are trimmed to nonzero products: conv1 taps drop the rows/
    cols whose input is a zero pad (center tap first carries start=True full
    coverage; a full dx=0 tap carries stop), and conv2's dy=+-1 DR pairs at
    dx=+-1 drop the one output column where both members read the pad
    column (~2.5 us of pure-zero PE work removed).
  * Software-pipelined emission (conv1 of image n+1 ahead of conv2 of n);
    act1 double-buffered as two static tiles with pad borders zeroed once;
    triple-buffered SBUF pools kill image-boundary WAR stalls; the round
    chain runs in image halves so conv2's early chunks unblock sooner;
    image 0's input DMA is split into row bands; the last output piece is
    a single 8-row chunk so the post-stream drain is short.  A post-Tile
    pass splits multi-semaphore waits onto same-engine NoOps (walrus
    encodes at most one sync wait per instruction).
  * Fixed overheads measured: ~8.5 us DMA-engine/queue startup before the
    first input byte moves, and a ~7 us NEFF exit epilogue (~60 rounds of
    EVENT_SEMAPHORE cleanup across all sequencers) — both invariant to
    kernel structure.  Occasional runs show ~17% uniform PE-clock
    throttling (external to the kernel).

  * PE p-state warm-up: ~30 short dummy matmuls on zeroed data run during
    the input-DMA wait, so image 0's matmuls start at the full 2.4 GHz
    clock instead of paying ~2.2 us of ramp (the tensor engine needs ~3 us
    of continuous work to reach max speed).

Measured (8 cores, NTFF profile): 169.2-172.3 us HW exec across runs
(baseline f32r/fp32 kernel: ~232 us on the same setup), rel L2 err 0.0106,
real-matmul stream ~150 us fully dense at mean 191.7 ns per 448-elem
matmul (189.1 is the quiet-engine floor; ~4 ns is SBUF port contention
from concurrent DVE/ScalarE traffic).
"""

import os
import sys

import numpy as np

for _p in ("/opt/trn_rl_repo", "/opt/pypackages"):
    if _p not in sys.path and os.path.isdir(_p):
        sys.path.insert(0, _p)

import ml_dtypes  # noqa: E402

# ---------------------------------------------------------------- constants
B, C, H, W = 64, 128, 56, 56
N_CORES = 8
BPC = B // N_CORES          # images per core
RPC = 8                     # output rows per PSUM chunk
NCHUNK = H // RPC           # 7 chunks
FREE = RPC * W              # 448 PSUM elems per chunk
XP = H + 2                  # padded x row length (58)
XPLANE = XP * XP            # 3364 fp16 elems per padded x plane
AW = 64                     # act1 padded row pitch (bytes, fp8)
AIMG = (H + 2) * AW         # 3712 padded act1 plane bytes
AB = 1                      # act1 base offset: keeps round-op dst offsets even
ACT_D = 3726                # shifted act1 copy offset; pair step +2 % 16 == 0
ATW = ACT_D + AB + AIMG     # act1 tile width (orig + shifted copy)
PLANE = H * W               # 3136
QUAD = PLANE // 4           # 784 elems per output quarter
MAGIC = float(2**23)        # fp32 round-to-nearest-even magic constant
EPS = 1e-5

_CACHE = {}


# ---------------------------------------------------------------- host math
def _quant_weight_int(w):
    """Return 15*quantize_weight(w, 4): exact odd integers in [-15, 15]."""
    wt = np.tanh(w.astype(np.float64)).astype(np.float32)
    m = np.float32(np.abs(wt).max())
    wtn = wt / (np.float32(2.0) * m) + np.float32(0.5)
    q = np.round(wtn * np.float32(15.0)).astype(np.float32)
    return np.float32(2.0) * q - np.float32(15.0)


def _bn_affine(gamma, beta, mean, var):
    inv = 1.0 / np.sqrt(var.astype(np.float64) + EPS)
    s = gamma.astype(np.float64) * inv
    b = beta.astype(np.float64) - mean.astype(np.float64) * s
    return s, b


def _lhsT_taps(w_int):
    """[oc, ic, 3, 3] -> [ic, 9*oc] stationary layout, tap-major."""
    t = np.transpose(w_int, (2, 3, 1, 0)).reshape(9, C, C)   # [tap, ic, oc]
    return np.transpose(t, (1, 0, 2)).reshape(C, 9 * C)


# ---------------------------------------------------------------- bass build
def _split_multiwaits(nc, mybir):
    """Walrus encodes at most ONE sync wait per instruction; hoist extras
    onto same-engine NoOps placed immediately before."""
    nid = 0
    for fn in nc.m.functions:
        for blk in fn.blocks:
            out = []
            changed = False
            for ins in blk.instructions:
                si = ins.sync_info
                if si is not None and len(si.on_wait) > 1:
                    waits = list(si.on_wait)
                    for w in waits[:-1]:
                        nid += 1
                        nop = mybir.InstNoOp(name=f"I-wfix-{nid}",
                                             engine=ins.engine)
                        nop.sync_info = mybir.SyncInfo(on_wait=[w],
                                                       on_update=[])
                        out.append(nop)
                    ins.sync_info = mybir.SyncInfo(
                        on_wait=[waits[-1]], on_update=list(si.on_update))
                    changed = True
                out.append(ins)
            if changed:
                blk.instructions = out


def _build_module(apply_wfix=True):
    import concourse.bass as bass
    import concourse.mybir as mybir
    import concourse.tile as tile
    from contextlib import ExitStack

    f32 = mybir.dt.float32
    f16 = mybir.dt.float16
    f8 = mybir.dt.float8e4
    AF = mybir.ActivationFunctionType
    OP = mybir.AluOpType
    DR = mybir.MatmulPerfMode.DoubleRow

    nc = bass.Bass("TRN2", target_bir_lowering=False, debug=False,
                   num_devices=N_CORES)

    xf_d = nc.dram_tensor("xf", [BPC, C, XPLANE], f16, kind="ExternalInput")
    w1_d = nc.dram_tensor("w1p", [C, 9 * C], f16, kind="ExternalInput")
    w2_d = nc.dram_tensor("w2p", [C, 9 * C], f8, kind="ExternalInput")
    bn_d = nc.dram_tensor("bnv", [C, 4], f32, kind="ExternalInput")
    out_d = nc.dram_tensor("out", [BPC, C, PLANE], f8, kind="ExternalOutput")

    with tile.TileContext(nc) as tc, ExitStack() as ctx:
        const = ctx.enter_context(tc.tile_pool(name="const", bufs=1))
        sb = ctx.enter_context(tc.tile_pool(name="sb", bufs=3))
        ps = ctx.enter_context(tc.tile_pool(name="ps", bufs=4, space="PSUM"))

        # conv1 weights first: they gate image 0's first matmul
        w1_sb = const.tile([C, 9 * C], f16)
        nc.sync.dma_start(w1_sb[:], w1_d.ap())
        bn_sb = const.tile([C, 4], f32)
        w2_sb = const.tile([C, 9 * C], f8)
        sc1, bi1 = bn_sb[:, 0:1], bn_sb[:, 1:2]
        sc2, bi2 = bn_sb[:, 2:3], bn_sb[:, 3:4]

        # PE p-state warm-up: the tensor engine needs ~3us of continuous
        # work to reach 2.4GHz; run short dummy matmuls on zeroed data
        # during the input-DMA wait so image 0 starts at full clock
        warm = const.tile([C, 512], f8)
        nc.gpsimd.memset(warm[:], 0.0)
        wps = ps.tile([C, 112], f32, tag="p2", name="warm_ps",
                      bufs=3)
        for k in range(30):
            mv = bass.AP(tensor=warm.tensor, offset=0,
                         ap=[[512, C], [1, 112]])
            nc.tensor.matmul(wps[:], lhsT=warm[:, 128:256], rhs=mv,
                             start=(k == 0), stop=(k == 29))

        # two static act1 slots; pad borders zeroed once
        act1a = const.tile([C, ATW], f8)
        act1b = const.tile([C, ATW], f8)
        for a1 in (act1a, act1b):
            for base in (AB, ACT_D + AB):
                r = a1[:, base:base + AIMG].rearrange("p (h w) -> p h w", w=AW)
                nc.gpsimd.memset(r[:, 0:1, :], 0.0)        # pad row 0
                nc.gpsimd.memset(r[:, 57:58, :], 0.0)      # pad row 57
                nc.gpsimd.memset(r[:, 1:57, 0], 0.0)       # pad col 0
                nc.gpsimd.memset(r[:, 1:57, 57:64], 0.0)   # dead cols

        def emit_load_conv1(n):
            """Load image n, conv1 + bn1 + qact; returns (xf, act1-slot)."""
            xf = sb.tile([C, XPLANE], f16, tag="xf", name=f"xf_{n}")
            if n == 0:
                # 4 row bands so chunk 0's matmuls start ~3us earlier
                for b0 in range(0, XPLANE, 16 * XP):
                    b1 = min(b0 + 16 * XP, XPLANE)
                    nc.sync.dma_start(xf[:, b0:b1], xf_d.ap()[n][:, b0:b1])
                # deferred const loads: off image 0's first-matmul path
                nc.sync.dma_start(bn_sb[:], bn_d.ap())
                nc.sync.dma_start(w2_sb[:], w2_d.ap())
            else:
                nc.sync.dma_start(xf[:], xf_d.ap()[n])

            v1 = sb.tile([C, PLANE], f16, tag="v1", name=f"v1_{n}")
            a1 = act1a if n % 2 == 0 else act1b

            for cch in range(NCHUNK):
                r0 = RPC * cch
                if cch == 5:
                    emit_round_half(n, v1, a1, 0)
                p1 = ps.tile([C, FREE], f32, tag="p1", name=f"p1_{n}_{cch}",
                             bufs=5)
                # center tap first (start, full 448); a full-coverage dx=0
                # tap last (stop); edge taps trimmed to nonzero products
                last = (1, 0) if cch < NCHUNK - 1 else (-1, 0)
                taps = [(0, 0)] + [(dy, dx) for dy in (-1, 0, 1)
                                   for dx in (-1, 0, 1)
                                   if (dy, dx) != (0, 0) and (dy, dx) != last]
                taps.append(last)
                for i, (dy, dx) in enumerate(taps):
                    t9 = (dy + 1) * 3 + (dx + 1)
                    rlo = max(r0, -dy)
                    rhi = min(r0 + RPC - 1, H - 1 - dy)
                    nr = rhi - rlo + 1
                    j0 = 0 if dx >= 0 else 1
                    nj = W - abs(dx)
                    off = (rlo + dy + 1) * XP + 1 + j0 + dx
                    mv = bass.AP(tensor=xf.tensor, offset=off,
                                 ap=[[XPLANE, C], [XP, nr], [1, nj]])
                    out = bass.AP(tensor=p1.tensor,
                                  offset=(rlo - r0) * W + j0,
                                  ap=[[FREE, C], [W, nr], [1, nj]])
                    nc.tensor.matmul(out, lhsT=w1_sb[:, t9 * C:(t9 + 1) * C],
                                     rhs=mv, start=(i == 0), stop=(i == 8))
                # bn1: Relu(s*psum+b) -> v1 fp16 (lower clip for free)
                nc.scalar.activation(v1[:, FREE * cch:FREE * (cch + 1)],
                                     p1[:], AF.Relu, bias=bi1, scale=sc1)

            return v1, xf, a1

        def emit_round_half(n, v1, a1, h):
            # upper clip + round-to-int into both fp8 act1 copies, by half:
            # conv2's early chunks unblock as soon as half 0 lands
            r0, r1 = (0, 28) if h == 0 else (28, 56)
            sl = slice(W * r0, W * r1)
            nc.vector.tensor_scalar_min(v1[:, sl], v1[:, sl], 15.0)
            v1r = v1[:].rearrange("p (h w) -> p h w", w=W)
            for base in (AB, ACT_D + AB):
                ar = a1[:, base:base + AIMG].rearrange("p (h w) -> p h w",
                                                       w=AW)
                nc.vector.tensor_scalar(ar[:, r0 + 1:r1 + 1, 1:57],
                                        v1r[:, r0:r1, :], MAGIC, MAGIC,
                                        op0=OP.add, op1=OP.subtract)

        def emit_conv2_out(n, xf, a1):
            """conv2 + bn2 + residual + qact for image n, DMA fp8 ints out."""
            v2 = sb.tile([C, PLANE], f16, tag="v2", name=f"v2_{n}")
            ost = sb.tile([C, PLANE], f8, tag="ost", name=f"ost_{n}")
            od = out_d.ap()[n]
            xfr = xf[:].rearrange("p (h w) -> p h w", w=XP)

            def emit_piece(r0, r1):
                sl = slice(W * r0, W * r1)
                nc.vector.tensor_tensor(v2[:, sl], v2[:, sl],
                                        xfr[:, r0 + 1:r1 + 1, 1:57],
                                        op=OP.add)
                nc.vector.tensor_scalar(v2[:, sl], v2[:, sl], MAGIC, MAGIC,
                                        op0=OP.add, op1=OP.subtract)
                nc.vector.tensor_scalar(ost[:, sl], v2[:, sl], 0.0, 15.0,
                                        op0=OP.max, op1=OP.min)
                nc.sync.dma_start(od[:, sl], ost[:, sl])

            # last piece is a single chunk so the post-stream drain is short
            piece_after = {1: (0, 14), 3: (14, 28), 5: (28, 48), 6: (48, 56)}
            for cch in range(NCHUNK):
                r0 = RPC * cch
                p2 = ps.tile([C, FREE], f32, tag="p2", name=f"p2_{n}_{cch}",
                             bufs=3)
                # center single (start, full coverage)
                mv = bass.AP(tensor=a1.tensor,
                             offset=AB + (r0 + 1) * AW + 1,
                             ap=[[ATW, C], [AW, RPC], [1, W]])
                nc.tensor.matmul(p2[:], lhsT=w2_sb[:, 8 * C:9 * C], rhs=mv,
                                 start=True, stop=False)
                # (dy=-1,dx)+(dy=+1,dx) DR pairs, stride 2*AW = 128B;
                # for dx=+-1 both members read the same zero pad column, so
                # that output column is trimmed
                for dxi, dx in enumerate((-1, 0, 1)):
                    j0 = 0 if dx >= 0 else 1
                    nj = W - abs(dx)
                    mv = bass.AP(tensor=a1.tensor,
                                 offset=AB + r0 * AW + 1 + j0 + dx,
                                 ap=[[ATW, C], [2 * AW, 2], [AW, RPC],
                                     [1, nj]])
                    wpair = w2_sb[:, dxi * 2 * C:(dxi + 1) * 2 * C].rearrange(
                        "p (two m) -> p two m", two=2)
                    out = bass.AP(tensor=p2.tensor, offset=j0,
                                  ap=[[FREE, C], [W, RPC], [1, nj]])
                    nc.tensor.matmul(out, lhsT=wpair, rhs=mv, perf_mode=DR,
                                     start=False, stop=False)
                # (dy=0,dx=-1)@orig + (dy=0,dx=+1)@shifted, stride ACT_D+2
                mv = bass.AP(tensor=a1.tensor, offset=AB + (r0 + 1) * AW,
                             ap=[[ATW, C], [ACT_D + 2, 2], [AW, RPC],
                                 [1, W]])
                wpair = w2_sb[:, 6 * C:8 * C].rearrange(
                    "p (two m) -> p two m", two=2)
                nc.tensor.matmul(p2[:], lhsT=wpair, rhs=mv, perf_mode=DR,
                                 start=False, stop=True)
                nc.scalar.activation(v2[:, FREE * cch:FREE * (cch + 1)],
                                     p2[:], AF.Identity, bias=bi2, scale=sc2)
                if cch in piece_after:
                    emit_piece(*piece_after[cch])

        prev = None
        for s in range(BPC + 1):
            cur = emit_load_conv1(s) if s < BPC else None
            if cur is not None:
                emit_round_half(s, cur[0], cur[2], 1)
            if prev is not None:
                emit_conv2_out(s - 1, prev[1], prev[2])
            prev = cur

    if apply_wfix:
        _split_multiwaits(nc, mybir)
    return nc


def _get_module(apply_wfix=True):
    key = ("nc", apply_wfix)
    if key not in _CACHE:
        _CACHE[key] = _build_module(apply_wfix)
    return _CACHE[key]


# ---------------------------------------------------------------- host entry
def _make_in_maps(x, w1, w2, gamma1, beta1, mean1, var1,
                  gamma2, beta2, mean2, var2):
    F8 = ml_dtypes.float8_e4m3
    x15 = np.float32(15.0) * np.asarray(x, np.float32)
    x15 = x15.reshape(N_CORES, BPC, C, H, W)

    # padded fp16 plane: rows/cols 1..56 live, zero borders
    xf = np.zeros((N_CORES, BPC, C, XP, XP), np.float16)
    xf[..., 1:57, 1:57] = x15.astype(np.float16)
    xf = xf.reshape(N_CORES, BPC, C, XPLANE)

    w1i = _quant_weight_int(np.asarray(w1, np.float32))
    w2i = _quant_weight_int(np.asarray(w2, np.float32))
    w1p = _lhsT_taps(w1i).astype(np.float16)
    w2t = _lhsT_taps(w2i)
    tap2 = lambda t9: w2t[:, t9 * C:(t9 + 1) * C]
    # conv2: 3 (dy=-1,dx)+(dy=+1,dx) pairs, the dy=0 dx=-1/+1 pair, center
    blocks = []
    for dxi in range(3):
        blocks += [tap2(dxi), tap2(6 + dxi)]
    blocks += [tap2(3), tap2(5), tap2(4)]
    w2p = np.concatenate(blocks, axis=1).astype(F8)

    s1, b1 = _bn_affine(np.asarray(gamma1, np.float32),
                        np.asarray(beta1, np.float32),
                        np.asarray(mean1, np.float32),
                        np.asarray(var1, np.float32))
    s2, b2 = _bn_affine(np.asarray(gamma2, np.float32),
                        np.asarray(beta2, np.float32),
                        np.asarray(mean2, np.float32),
                        np.asarray(var2, np.float32))
    # PSUM holds 225*conv (15x and 15w) -> affine to 15*bn
    bnv = np.stack([s1 / 15.0, 15.0 * b1, s2 / 15.0, 15.0 * b2],
                   axis=1).astype(np.float32)

    shared = {"w1p": w1p, "w2p": w2p, "bnv": bnv}
    return [{"xf": np.ascontiguousarray(xf[i]), **shared}
            for i in range(N_CORES)]


def kernel(**inputs):
    from concourse.bass_utils import run_bass_kernel_spmd

    nc = _get_module()
    in_maps = _make_in_maps(**inputs)
    res = run_bass_kernel_spmd(nc, in_maps, core_ids=list(range(N_CORES)))
    _CACHE["last_res"] = res
    # exact: out fp8 ints k in 0..15 -> f32 k/15
    out = np.concatenate(
        [np.asarray(r["out"]).astype(np.float32) / np.float32(15.0)
         for r in res.results], axis=0)
    return out.reshape(B, C, H, W)
